# revision 21
# baseline (speedup 1.0000x reference)
"""FP8-weight dense linear (FFN up-proj) on 8 Trainium2 NeuronCores.

Computes out[128, 16384] = x[128, 4096] @ dequant(weight_fp8[16384, 4096]).T
+ bias, tensor-parallel: weight/bias sharded along out_features (2048 rows
per core), x replicated, output gathered by concatenation (no collectives).

Per-core kernel design (v2 — DoubleRow fp8):
- The PE's fp8 DoubleRow mode contracts 256 elements/column (2 fp8 rows per
  partition) at 1 column/cycle — 2x the MAC rate of the fp16-stationary
  path. Both operands must be fp8.
- x is quantized to fp8e4 on the host as xq plus a quantized residual
  rq = fp8(x - xq); the device accumulates xq@w for all k and rq@w for the
  first COMP_JT k-chunks into the same PSUM group (the residual pass
  re-reads the same SBUF weight bytes, costing only PE time). Combined
  x error ~2^-8 relative on compensated chunks; the COMP_JT knob trades
  PE time vs quantization error (calibrated against the 2e-2 gate).
- The weight shard is prepacked on the host into the exact SBUF layout
  [jt, p, (osl o two)] so the device DMA is a pure linear per-partition
  stream (large descriptors, full HBM bandwidth) instead of the 256B/packet
  DMA-transpose xbar path, and the DoubleRow moving AP is just
  rearrange("p (o two) -> p two o") on a contiguous row.
- Bias is added during PSUM eviction on the vector engine (broadcast along
  partitions), not via fp32 matmuls.
"""

import sys

if "/opt/trn_rl_repo" not in sys.path:
    sys.path.insert(0, "/opt/trn_rl_repo")

import numpy as np
import ml_dtypes

import concourse.bass as bass  # noqa: F401  (registers bass lowering)
import concourse.mybir as mybir
import concourse.tile as tile
from concourse import bacc

N_CORES = 8
T = 128          # tokens
K = 4096         # in_features
O_FULL = 16384   # out_features
O = O_FULL // N_CORES  # 2048 per core
O_CHUNK = 512    # psum bank / matmul free dim
N_OSL = O // O_CHUNK   # 4 o-slices per core
JT = K // 256    # 16 k-chunks of 256 (one DoubleRow MM each)
COMP_JT = 10     # k-chunks with residual compensation (error knob)

_E4M3 = ml_dtypes.float8_e4m3


N_G = 4          # w chunks per o-slice (each = JT/N_G jt of one osl)
JT_G = JT // N_G


def _build_nc(repeats: int = 1, comp_jt: int = COMP_JT):
    """Per-core BIR. repeats>1 wraps the body in a hardware For_i loop
    (benchmark-only). The weight streams per o-slice (osl-major) so each
    PSUM group closes as soon as its own 2MB arrives; evict + store of
    osl overlap the next osl's matmuls."""
    assert 0 <= comp_jt <= JT
    # num_devices=1: the kernel is fully SPMD-symmetric (inputs pre-sharded
    # on host, no collectives, partition id never read) — skips the
    # partition-id load + one preamble barrier round in the NEFF.
    nc = bacc.Bacc("TRN2", target_bir_lowering=False, debug=False,
                   num_devices=1)
    w_d = nc.dram_tensor("wlin", [N_OSL, N_G, 128, JT_G * O_CHUNK * 2],
                         mybir.dt.uint8, kind="ExternalInput")
    xq_d = nc.dram_tensor("xq", [128, JT * 2 * T], mybir.dt.uint8,
                          kind="ExternalInput")
    rq_d = nc.dram_tensor("rq", [128, max(comp_jt, 1) * 2 * T],
                          mybir.dt.uint8, kind="ExternalInput")
    b_d = nc.dram_tensor("bias", [1, O], mybir.dt.float32,
                         kind="ExternalInput")
    o_d = nc.dram_tensor("out", [T, O], mybir.dt.float16,
                         kind="ExternalOutput")

    with tile.TileContext(nc) as tc:
        with (
            tc.tile_pool(name="const", bufs=1) as const,
            tc.tile_pool(name="xpool", bufs=1) as xpool,
            tc.tile_pool(name="wpool", bufs=1) as wpool,
            tc.tile_pool(name="opool", bufs=1) as opool,
            tc.tile_pool(name="psum", bufs=1, space="PSUM") as psum,
        ):
            def body():
                # Two parallel DMA issue streams: GpSimd (SWDGE) carries
                # xq/rq/bias + output stores; the Sync engine (HWDGE,
                # ~0.7us serial issue each) carries only weight chunks,
                # so the 8MB weight stream starts immediately.
                xq = xpool.tile([128, JT, 2, T], mybir.dt.uint8)
                xq_flat = xq[:].rearrange("p jt two t -> p (jt two t)")
                rq = xpool.tile([128, max(comp_jt, 1), 2, T], mybir.dt.uint8)
                rq_flat = rq[:].rearrange("p jt two t -> p (jt two t)")
                sl = 2 * 2 * T  # 2-jt sliver of xq/rq

                # Sync carries the 2-jt x/r slivers then the weight chunks;
                # GpSimd (SWDGE) carries the xq/rq remainders + bias.
                nc.sync.dma_start(xq_flat[:, :sl], xq_d.ap()[:, :sl])
                if comp_jt > 0:
                    nc.sync.dma_start(rq_flat[:, :sl], rq_d.ap()[:, :sl])
                nc.gpsimd.dma_start(xq_flat[:, sl:], xq_d.ap()[:, sl:])
                if comp_jt > 2:
                    nc.gpsimd.dma_start(rq_flat[:, sl:], rq_d.ap()[:, sl:])
                bias_sb = const.tile([1, O], mybir.dt.float32)
                nc.gpsimd.dma_start(bias_sb[:], b_d.ap())
                bias_full = const.tile([T, O], mybir.dt.float32)
                nc.gpsimd.partition_broadcast(bias_full[:], bias_sb[:])

                wt = wpool.tile([128, N_OSL, N_G, JT_G * O_CHUNK * 2],
                                mybir.dt.uint8)

                def w_dma(osl, g):
                    nc.sync.dma_start(wt[:, osl, g, :], w_d.ap()[osl, g])

                half = 2 * O_CHUNK * 2  # 2 jt of one chunk
                for g in range(N_G):  # osl0 in 2-jt chunks (ramp phase)
                    nc.sync.dma_start(wt[:, 0, g, :half],
                                      w_d.ap()[0, g][:, :half])
                    nc.sync.dma_start(wt[:, 0, g, half:],
                                      w_d.ap()[0, g][:, half:])
                for osl in range(1, N_OSL):
                    for g in range(N_G):
                        w_dma(osl, g)

                xq8 = xq[:].bitcast(mybir.dt.float8e4)
                rq8 = rq[:].bitcast(mybir.dt.float8e4)
                wt8 = wt[:].bitcast(mybir.dt.float8e4).rearrange(
                    "p osl g (jtl o two) -> p osl g jtl o two",
                    jtl=JT_G, o=O_CHUNK, two=2)

                pss = [psum.tile([T, O_CHUNK], mybir.dt.float32,
                                 name=f"ps{osl}", tag=f"ps{osl}")
                       for osl in range(N_OSL)]
                out_sb = opool.tile([T, O], mybir.dt.float16)

                def issue(jt, osl, stop=False):
                    g, jtl = jt // JT_G, jt % JT_G
                    mov = wt8[:, osl, g, jtl].rearrange("p o two -> p two o")
                    comp = jt < comp_jt
                    nc.tensor.matmul(pss[osl][:], xq8[:, jt], mov,
                                     start=(jt == 0),
                                     stop=(stop and not comp),
                                     perf_mode=mybir.MatmulPerfMode.DoubleRow)
                    if comp:
                        nc.tensor.matmul(
                            pss[osl][:], rq8[:, jt], mov,
                            start=False, stop=stop,
                            perf_mode=mybir.MatmulPerfMode.DoubleRow)

                def finish(osl):
                    nc.vector.tensor_tensor(
                        out_sb[:, osl * O_CHUNK:(osl + 1) * O_CHUNK],
                        pss[osl][:],
                        bias_full[:, osl * O_CHUNK:(osl + 1) * O_CHUNK],
                        op=mybir.AluOpType.add)
                    nc.sync.dma_start(
                        o_d.ap()[:, osl * O_CHUNK:(osl + 1) * O_CHUNK],
                        out_sb[:, osl * O_CHUNK:(osl + 1) * O_CHUNK])

                for osl in range(N_OSL):
                    for jt in range(JT):
                        issue(jt, osl, stop=(jt == JT - 1))
                    finish(osl)

            if repeats == 1:
                body()
            else:
                with tc.For_i(0, repeats, 1):
                    body()

    nc.compile()
    return nc


BEST_CONFIG = dict(comp_jt=COMP_JT)

_NC = None


def _get_nc():
    global _NC
    if _NC is None:
        _NC = _build_nc(**BEST_CONFIG)
    return _NC


_FN = None


def _get_fn():
    """Cache the jitted SPMD callable so repeat kernel() calls skip the
    ~1.3s of re-tracing that run_bass_kernel_spmd pays per invocation."""
    global _FN
    if _FN is not None:
        return _FN
    import jax
    from jax.sharding import Mesh, PartitionSpec, NamedSharding
    from jax.experimental.shard_map import shard_map
    from concourse import bass2jax as b2j

    nc = _get_nc()
    b2j.install_neuronx_cc_hook()
    pname = nc.partition_id_tensor.name if nc.partition_id_tensor else None
    in_names, out_names, out_avals = [], [], []
    for alloc in nc.m.functions[0].allocations:
        if not isinstance(alloc, mybir.MemoryLocationSet):
            continue
        name = alloc.memorylocations[0].name
        if alloc.kind == "ExternalInput":
            if name != pname:
                in_names.append(name)
        elif alloc.kind == "ExternalOutput":
            out_names.append(name)
            out_avals.append(jax.core.ShapedArray(
                tuple(alloc.tensor_shape), mybir.dt.np(alloc.dtype)))
    n_params, n_outs = len(in_names), len(out_avals)
    all_in = in_names + out_names + ([pname] if pname else [])

    def _body(*args):
        operands = list(args)
        if pname:
            operands.append(b2j.partition_id_tensor())
        outs = b2j._bass_exec_p.bind(
            *operands, out_avals=tuple(out_avals), in_names=tuple(all_in),
            out_names=tuple(out_names), lowering_input_output_aliases=(),
            sim_require_finite=True, sim_require_nnan=True, nc=nc)
        return tuple(outs)

    mesh = Mesh(np.asarray(jax.devices()[:N_CORES]), ("core",))
    fn = jax.jit(shard_map(_body, mesh=mesh,
                           in_specs=(PartitionSpec("core"),) * (n_params + n_outs),
                           out_specs=(PartitionSpec("core"),) * n_outs,
                           check_rep=False), keep_unused=True)
    sharding = NamedSharding(mesh, PartitionSpec("core"))
    _FN = (fn, in_names, out_avals, sharding)
    return _FN


def _prep_inputs(x, w, b, comp_jt=COMP_JT):
    """Host-side marshaling: shard + prepack into the device layouts.

    wlin[n, osl, g, p, (jtl o two)] =
        w_u8[n*O + osl*512 + o, ((g*JT_G+jtl)*128+p)*2 + two]
    xq/rq [p, (jt two t)] = fp8(x)[t, jt*256 + p*2 + two] (and residual).
    """
    w_u8 = np.ascontiguousarray(w).view(np.uint8)  # [16384, 4096]
    ws = w_u8.reshape(N_CORES, N_OSL, O_CHUNK, N_G, JT_G, 128, 2)
    wlin = np.ascontiguousarray(ws.transpose(0, 1, 3, 5, 4, 2, 6)).reshape(
        N_CORES, N_OSL, N_G, 128, JT_G * O_CHUNK * 2)

    xq = x.astype(_E4M3)
    r = x - xq.astype(np.float32)
    rq = r.astype(_E4M3)

    def xpack(a8, njt):
        at = a8.view(np.uint8).reshape(T, JT, 128, 2)[:, :njt]
        return np.ascontiguousarray(
            at.transpose(2, 1, 3, 0)).reshape(128, njt * 2 * T)

    return {
        "wlin": wlin,  # sharded along axis 0 by core
        "xq": np.broadcast_to(xpack(xq, JT), (N_CORES, 128, JT * 2 * T)),
        "rq": np.broadcast_to(
            xpack(rq, max(comp_jt, 1)),
            (N_CORES, 128, max(comp_jt, 1) * 2 * T)),
        "bias": b.reshape(N_CORES, 1, O),
    }


def kernel(x, weight_fp8, bias):
    import jax
    x = np.ascontiguousarray(np.asarray(x), dtype=np.float32)
    w = np.ascontiguousarray(np.asarray(weight_fp8))
    b = np.ascontiguousarray(np.asarray(bias), dtype=np.float32)
    assert x.shape == (T, K) and w.shape == (O_FULL, K)

    fn, in_names, out_avals, sharding = _get_fn()
    per_core = _prep_inputs(x, w, b, BEST_CONFIG["comp_jt"])
    per_core = {k: v.reshape(-1, *v.shape[2:]) for k, v in per_core.items()}
    dev_in = [jax.device_put(np.ascontiguousarray(per_core[n]), sharding)
              for n in in_names]
    dev_zero = [jax.device_put(
        np.zeros((N_CORES * a.shape[0], *a.shape[1:]), a.dtype), sharding)
        for a in out_avals]
    outs = fn(*dev_in, *dev_zero)
    res = np.asarray(jax.device_get(outs[0])).reshape(N_CORES, T, O)
    return np.concatenate(list(res), axis=1).astype(np.float32)


# revision 25
# speedup vs baseline: 1.0080x; 1.0080x over previous
"""FP8-weight dense linear (FFN up-proj) on 8 Trainium2 NeuronCores.

Computes out[128, 16384] = x[128, 4096] @ dequant(weight_fp8[16384, 4096]).T
+ bias, tensor-parallel: weight/bias sharded along out_features (2048 rows
per core), x replicated, output gathered by concatenation (no collectives).

Per-core kernel design (v2 — DoubleRow fp8):
- The PE's fp8 DoubleRow mode contracts 256 elements/column (2 fp8 rows per
  partition) at 1 column/cycle — 2x the MAC rate of the fp16-stationary
  path. Both operands must be fp8.
- x is quantized to fp8e4 on the host as xq plus a quantized residual
  rq = fp8(x - xq); the device accumulates xq@w for all k and rq@w for the
  first COMP_JT k-chunks into the same PSUM group (the residual pass
  re-reads the same SBUF weight bytes, costing only PE time). Combined
  x error ~2^-8 relative on compensated chunks; the COMP_JT knob trades
  PE time vs quantization error (calibrated against the 2e-2 gate).
- The weight shard is prepacked on the host into the exact SBUF layout
  [jt, p, (osl o two)] so the device DMA is a pure linear per-partition
  stream (large descriptors, full HBM bandwidth) instead of the 256B/packet
  DMA-transpose xbar path, and the DoubleRow moving AP is just
  rearrange("p (o two) -> p two o") on a contiguous row.
- Bias is added during PSUM eviction on the vector engine (broadcast along
  partitions), not via fp32 matmuls.
"""

import sys

if "/opt/trn_rl_repo" not in sys.path:
    sys.path.insert(0, "/opt/trn_rl_repo")

import numpy as np
import ml_dtypes

import concourse.bass as bass  # noqa: F401  (registers bass lowering)
import concourse.mybir as mybir
import concourse.tile as tile
from concourse import bacc

N_CORES = 8
T = 128          # tokens
K = 4096         # in_features
O_FULL = 16384   # out_features
O = O_FULL // N_CORES  # 2048 per core
O_CHUNK = 512    # psum bank / matmul free dim
N_OSL = O // O_CHUNK   # 4 o-slices per core
JT = K // 256    # 16 k-chunks of 256 (one DoubleRow MM each)
COMP_JT = 9      # k-chunks with residual compensation (error knob):
                 # true-input max-rel 1.734e-2 vs the 2e-2 gate

_E4M3 = ml_dtypes.float8_e4m3


N_G = 4          # w chunks per o-slice (each = JT/N_G jt of one osl)
JT_G = JT // N_G


def _build_nc(repeats: int = 1, comp_jt: int = COMP_JT):
    """Per-core BIR. repeats>1 wraps the body in a hardware For_i loop
    (benchmark-only). The weight streams per o-slice (osl-major) so each
    PSUM group closes as soon as its own 2MB arrives; evict + store of
    osl overlap the next osl's matmuls."""
    assert 0 <= comp_jt <= JT
    # num_devices=1: the kernel is fully SPMD-symmetric (inputs pre-sharded
    # on host, no collectives, partition id never read) — skips the
    # partition-id load + one preamble barrier round in the NEFF.
    nc = bacc.Bacc("TRN2", target_bir_lowering=False, debug=False,
                   num_devices=1)
    w_d = nc.dram_tensor("wlin", [N_OSL, N_G, 128, JT_G * O_CHUNK * 2],
                         mybir.dt.uint8, kind="ExternalInput")
    xq_d = nc.dram_tensor("xq", [128, JT * 2 * T], mybir.dt.uint8,
                          kind="ExternalInput")
    rq_d = nc.dram_tensor("rq", [128, max(comp_jt, 1) * 2 * T],
                          mybir.dt.uint8, kind="ExternalInput")
    b_d = nc.dram_tensor("bias", [1, O], mybir.dt.float32,
                         kind="ExternalInput")
    o_d = nc.dram_tensor("out", [T, O], mybir.dt.float16,
                         kind="ExternalOutput")

    with tile.TileContext(nc) as tc:
        with (
            tc.tile_pool(name="const", bufs=1) as const,
            tc.tile_pool(name="xpool", bufs=1) as xpool,
            tc.tile_pool(name="wpool", bufs=1) as wpool,
            tc.tile_pool(name="opool", bufs=1) as opool,
            tc.tile_pool(name="psum", bufs=1, space="PSUM") as psum,
        ):
            def body():
                # Two parallel DMA issue streams: GpSimd (SWDGE) carries
                # xq/rq/bias + output stores; the Sync engine (HWDGE,
                # ~0.7us serial issue each) carries only weight chunks,
                # so the 8MB weight stream starts immediately.
                xq = xpool.tile([128, JT, 2, T], mybir.dt.uint8)
                xq_flat = xq[:].rearrange("p jt two t -> p (jt two t)")
                rq = xpool.tile([128, max(comp_jt, 1), 2, T], mybir.dt.uint8)
                rq_flat = rq[:].rearrange("p jt two t -> p (jt two t)")
                sl = 4 * 2 * T  # 4-jt sliver of xq/rq

                # Sync carries the 2-jt x/r slivers then the weight chunks;
                # GpSimd (SWDGE) carries the xq/rq remainders + bias.
                nc.sync.dma_start(xq_flat[:, :sl], xq_d.ap()[:, :sl])
                if comp_jt > 0:
                    nc.sync.dma_start(rq_flat[:, :sl], rq_d.ap()[:, :sl])
                nc.gpsimd.dma_start(xq_flat[:, sl:], xq_d.ap()[:, sl:])
                if comp_jt > 4:
                    nc.gpsimd.dma_start(rq_flat[:, sl:], rq_d.ap()[:, sl:])
                bias_sb = const.tile([1, O], mybir.dt.float32)
                nc.gpsimd.dma_start(bias_sb[:], b_d.ap())
                bias_full = const.tile([T, O], mybir.dt.float32)
                nc.gpsimd.partition_broadcast(bias_full[:], bias_sb[:])

                wt = wpool.tile([128, N_OSL, N_G, JT_G * O_CHUNK * 2],
                                mybir.dt.uint8)

                def w_dma(osl, g):
                    nc.sync.dma_start(wt[:, osl, g, :], w_d.ap()[osl, g])

                half = 2 * O_CHUNK * 2  # 2 jt of one chunk
                for osl in range(2):  # osl0/1: 2-jt chunks (ramp phase)
                    for g in range(N_G):
                        nc.sync.dma_start(wt[:, osl, g, :half],
                                          w_d.ap()[osl, g][:, :half])
                        nc.sync.dma_start(wt[:, osl, g, half:],
                                          w_d.ap()[osl, g][:, half:])
                for osl in range(2, N_OSL):
                    for g in range(N_G):
                        w_dma(osl, g)

                xq8 = xq[:].bitcast(mybir.dt.float8e4)
                rq8 = rq[:].bitcast(mybir.dt.float8e4)
                wt8 = wt[:].bitcast(mybir.dt.float8e4).rearrange(
                    "p osl g (jtl o two) -> p osl g jtl o two",
                    jtl=JT_G, o=O_CHUNK, two=2)

                pss = [psum.tile([T, O_CHUNK], mybir.dt.float32,
                                 name=f"ps{osl}", tag=f"ps{osl}")
                       for osl in range(N_OSL)]
                out_sb = opool.tile([T, O], mybir.dt.float16)

                def issue(jt, osl, stop=False):
                    g, jtl = jt // JT_G, jt % JT_G
                    mov = wt8[:, osl, g, jtl].rearrange("p o two -> p two o")
                    comp = jt < comp_jt
                    nc.tensor.matmul(pss[osl][:], xq8[:, jt], mov,
                                     start=(jt == 0),
                                     stop=(stop and not comp),
                                     perf_mode=mybir.MatmulPerfMode.DoubleRow)
                    if comp:
                        nc.tensor.matmul(
                            pss[osl][:], rq8[:, jt], mov,
                            start=False, stop=stop,
                            perf_mode=mybir.MatmulPerfMode.DoubleRow)

                def finish(osl):
                    nc.vector.tensor_tensor(
                        out_sb[:, osl * O_CHUNK:(osl + 1) * O_CHUNK],
                        pss[osl][:],
                        bias_full[:, osl * O_CHUNK:(osl + 1) * O_CHUNK],
                        op=mybir.AluOpType.add)
                    nc.sync.dma_start(
                        o_d.ap()[:, osl * O_CHUNK:(osl + 1) * O_CHUNK],
                        out_sb[:, osl * O_CHUNK:(osl + 1) * O_CHUNK])

                for osl in range(N_OSL):
                    for jt in range(JT):
                        issue(jt, osl, stop=(jt == JT - 1))
                    finish(osl)

            if repeats == 1:
                body()
            else:
                with tc.For_i(0, repeats, 1):
                    body()

    nc.compile()
    return nc


BEST_CONFIG = dict(comp_jt=COMP_JT)

_NC = None


def _get_nc():
    global _NC
    if _NC is None:
        _NC = _build_nc(**BEST_CONFIG)
    return _NC


_FN = None


def _get_fn():
    """Cache the jitted SPMD callable so repeat kernel() calls skip the
    ~1.3s of re-tracing that run_bass_kernel_spmd pays per invocation."""
    global _FN
    if _FN is not None:
        return _FN
    import jax
    from jax.sharding import Mesh, PartitionSpec, NamedSharding
    from jax.experimental.shard_map import shard_map
    from concourse import bass2jax as b2j

    nc = _get_nc()
    b2j.install_neuronx_cc_hook()
    pname = nc.partition_id_tensor.name if nc.partition_id_tensor else None
    in_names, out_names, out_avals = [], [], []
    for alloc in nc.m.functions[0].allocations:
        if not isinstance(alloc, mybir.MemoryLocationSet):
            continue
        name = alloc.memorylocations[0].name
        if alloc.kind == "ExternalInput":
            if name != pname:
                in_names.append(name)
        elif alloc.kind == "ExternalOutput":
            out_names.append(name)
            out_avals.append(jax.core.ShapedArray(
                tuple(alloc.tensor_shape), mybir.dt.np(alloc.dtype)))
    n_params, n_outs = len(in_names), len(out_avals)
    all_in = in_names + out_names + ([pname] if pname else [])

    def _body(*args):
        operands = list(args)
        if pname:
            operands.append(b2j.partition_id_tensor())
        outs = b2j._bass_exec_p.bind(
            *operands, out_avals=tuple(out_avals), in_names=tuple(all_in),
            out_names=tuple(out_names), lowering_input_output_aliases=(),
            sim_require_finite=True, sim_require_nnan=True, nc=nc)
        return tuple(outs)

    mesh = Mesh(np.asarray(jax.devices()[:N_CORES]), ("core",))
    fn = jax.jit(shard_map(_body, mesh=mesh,
                           in_specs=(PartitionSpec("core"),) * (n_params + n_outs),
                           out_specs=(PartitionSpec("core"),) * n_outs,
                           check_rep=False), keep_unused=True)
    sharding = NamedSharding(mesh, PartitionSpec("core"))
    _FN = (fn, in_names, out_avals, sharding)
    return _FN


def _prep_inputs(x, w, b, comp_jt=COMP_JT):
    """Host-side marshaling: shard + prepack into the device layouts.

    wlin[n, osl, g, p, (jtl o two)] =
        w_u8[n*O + osl*512 + o, ((g*JT_G+jtl)*128+p)*2 + two]
    xq/rq [p, (jt two t)] = fp8(x)[t, jt*256 + p*2 + two] (and residual).
    """
    w_u8 = np.ascontiguousarray(w).view(np.uint8)  # [16384, 4096]
    ws = w_u8.reshape(N_CORES, N_OSL, O_CHUNK, N_G, JT_G, 128, 2)
    wlin = np.ascontiguousarray(ws.transpose(0, 1, 3, 5, 4, 2, 6)).reshape(
        N_CORES, N_OSL, N_G, 128, JT_G * O_CHUNK * 2)

    xq = x.astype(_E4M3)
    r = x - xq.astype(np.float32)
    rq = r.astype(_E4M3)

    def xpack(a8, njt):
        at = a8.view(np.uint8).reshape(T, JT, 128, 2)[:, :njt]
        return np.ascontiguousarray(
            at.transpose(2, 1, 3, 0)).reshape(128, njt * 2 * T)

    return {
        "wlin": wlin,  # sharded along axis 0 by core
        "xq": np.broadcast_to(xpack(xq, JT), (N_CORES, 128, JT * 2 * T)),
        "rq": np.broadcast_to(
            xpack(rq, max(comp_jt, 1)),
            (N_CORES, 128, max(comp_jt, 1) * 2 * T)),
        "bias": b.reshape(N_CORES, 1, O),
    }


def kernel(x, weight_fp8, bias):
    import jax
    x = np.ascontiguousarray(np.asarray(x), dtype=np.float32)
    w = np.ascontiguousarray(np.asarray(weight_fp8))
    b = np.ascontiguousarray(np.asarray(bias), dtype=np.float32)
    assert x.shape == (T, K) and w.shape == (O_FULL, K)

    fn, in_names, out_avals, sharding = _get_fn()
    per_core = _prep_inputs(x, w, b, BEST_CONFIG["comp_jt"])
    per_core = {k: v.reshape(-1, *v.shape[2:]) for k, v in per_core.items()}
    dev_in = [jax.device_put(np.ascontiguousarray(per_core[n]), sharding)
              for n in in_names]
    dev_zero = [jax.device_put(
        np.zeros((N_CORES * a.shape[0], *a.shape[1:]), a.dtype), sharding)
        for a in out_avals]
    outs = fn(*dev_in, *dev_zero)
    res = np.asarray(jax.device_get(outs[0])).reshape(N_CORES, T, O)
    return np.concatenate(list(res), axis=1).astype(np.float32)
